# revision 13
# baseline (speedup 1.0000x reference)
"""Grouped MoE (top-2 of 8 experts, SwiGLU) on 8 Trainium2 NeuronCores.

Expert-parallel with host routing (gate on host, exact). Core c owns
expert c; tokens are gathered per expert into a fixed-capacity [D, cap]
buffer. On device each core runs the three SwiGLU GEMMs in bf16 over its
~T*K/E tokens and writes an UNSCALED output in transposed [D, cap]
layout; the host applies the per-token gate weight and scatter-adds the
two expert contributions. No collectives.

v2 layout/schedule changes vs the first working kernel:
 - All DRAM inputs are packed partition-major so every DMA moves 2-16 KB
   contiguous rows (128 descriptors/transfer, full HBM rate). w1/w3 are
   packed in per-f-tile blocks so the first A-stage matmul only needs
   x-chunk0 + one 256 KB block instead of ~2 MB.
 - Y-stage is w2-stationary (output [D, tokens]): no partial m-tiles,
   arbitrary chunk sizes, and the gate scale moves to the host combine.
 - First/last chunks are small to shrink the DMA lead-in and the
   copy+store tail after the final matmul.
"""

import sys
import numpy as np

for _p in ("/opt/trn_rl_repo",):
    if _p not in sys.path:
        sys.path.insert(0, _p)

B, S, D, F, E, K = 2, 2048, 1024, 1024, 8, 2
T = B * S            # 4096 tokens
NCORES = 8
P = 128
DK = D // P          # 8 contraction chunks over D
FK = F // P          # 8 F tiles
BLK = DK * P         # w1/w3 f-block stride (k-major within a block)
NWARM = 12           # PE warm-up matmuls while the first DMAs land

_cache = {}


def _chunks(cap):
    """Token chunks <= 512 (PSUM bank limit), first/last kept small-ish.

    A/B and Y matmul cost is proportional to total tokens for any chunk
    >= ~192 (LDWEIGHTS hides under the column stream), so only the first
    chunk (gates the DMA lead-in) and last chunk (gates the tail) matter.
    """
    if cap <= 512:
        sizes = [cap]
    elif cap <= 832:
        sizes = [(cap + 1) // 2, cap // 2]
    else:
        sizes = [320]
        rem = cap - 320
        while rem > 704:
            sizes.append(512)
            rem -= 512
        if rem > 512:
            sizes += [(rem + 1) // 2, rem // 2]
        else:
            sizes.append(rem)
    out = []
    o = 0
    for s in sizes:
        out.append((o, s))
        o += s
    assert o == cap and all(0 < s <= 512 for _, s in out)
    return out


def _build_nc(cap):
    from contextlib import ExitStack

    import concourse.mybir as mybir
    import concourse.tile as tile
    from concourse import bacc

    dt = mybir.dt
    AF = mybir.ActivationFunctionType
    ALU = mybir.AluOpType

    chunks = _chunks(cap)

    nc = bacc.Bacc("TRN2", target_bir_lowering=False, debug=False,
                   num_devices=NCORES)

    # all partition-major: row p holds that partition's full data span
    xh = nc.dram_tensor("xh", [P, DK * cap], dt.bfloat16,
                        kind="ExternalInput").ap()
    w1h = nc.dram_tensor("w1h", [P, FK * BLK], dt.bfloat16,
                         kind="ExternalInput").ap()
    w3h = nc.dram_tensor("w3h", [P, FK * BLK], dt.bfloat16,
                         kind="ExternalInput").ap()
    w2h = nc.dram_tensor("w2h", [P, FK * D], dt.bfloat16,
                         kind="ExternalInput").ap()
    out = nc.dram_tensor("out", [D, cap], dt.bfloat16,
                         kind="ExternalOutput").ap()

    with tile.TileContext(nc) as tc, ExitStack() as ctx:
        const = ctx.enter_context(tc.tile_pool(name="const", bufs=1))
        xpool = ctx.enter_context(tc.tile_pool(name="xpool", bufs=1))
        spool = ctx.enter_context(tc.tile_pool(name="spool", bufs=2))
        hpool = ctx.enter_context(tc.tile_pool(name="hpool", bufs=2))
        ypool = ctx.enter_context(tc.tile_pool(name="ypool", bufs=3))

        abpsum = ctx.enter_context(tc.tile_pool(name="abpsum", bufs=2,
                                                space="PSUM"))
        ypsum = ctx.enter_context(tc.tile_pool(name="ypsum", bufs=4,
                                               space="PSUM"))

        x_sb = xpool.tile([P, DK * cap], dt.bfloat16, tag="xall")
        w1_sb = const.tile([P, FK * BLK], dt.bfloat16, tag="w1")
        w3_sb = const.tile([P, FK * BLK], dt.bfloat16, tag="w3")
        w2_sb = const.tile([P, FK * D], dt.bfloat16, tag="w2")

        # ---- DMA issue order = criticality order. Every transfer below is
        # 128 descriptors of >= 2KB contiguous rows (full HBM rate).
        # First A-group (f=0) is gated only on x-chunk0 + w1/w3 f0 blocks
        # (~1 MB); remaining f-blocks stream in ahead of the compute. ----
        # All DMA on the sync queue: its sequencer starts issuing at ~0.1us
        # while the other engines only come online at 5-8us, so anything
        # issued elsewhere would land BEHIND these transfers in the rings.
        o0, tc0 = chunks[0]
        nc.sync.dma_start(x_sb[:, 0:DK * tc0], xh[:, 0:DK * tc0])
        nc.sync.dma_start(w1_sb[:, 0:BLK], w1h[:, 0:BLK])
        nc.sync.dma_start(w3_sb[:, 0:BLK], w3h[:, 0:BLK])
        for f in range(1, FK):
            nc.sync.dma_start(w1_sb[:, f * BLK:(f + 1) * BLK],
                              w1h[:, f * BLK:(f + 1) * BLK])
            nc.sync.dma_start(w3_sb[:, f * BLK:(f + 1) * BLK],
                              w3h[:, f * BLK:(f + 1) * BLK])
        # w2 in fk-halves; the Y loop consumes fk 0..3 before 4..7
        nc.sync.dma_start(w2_sb[:, 0:4 * D], w2h[:, 0:4 * D])
        nc.sync.dma_start(w2_sb[:, 4 * D:8 * D], w2h[:, 4 * D:8 * D])
        for (o, tcz) in chunks[1:]:
            nc.sync.dma_start(x_sb[:, DK * o:DK * (o + tcz)],
                              xh[:, DK * o:DK * (o + tcz)])

        # ---- PE warm-up: dummy matmuls while the first DMAs land keep the
        # HAM activity window full so the PE reaches max p-state ----
        wrm = spool.tile([P, 512], dt.bfloat16, tag="wrm")
        nc.vector.memset(wrm[:], 0.5)
        for _ in range(NWARM):
            psW = abpsum.tile([P, 512], dt.float32, tag="psA", name="psW")
            nc.tensor.matmul(psW[:], lhsT=wrm[:, 0:P], rhs=wrm[:],
                             start=True, stop=True)

        # ---- per-chunk SwiGLU FFN ----
        for (o, tcz) in chunks:
            xo = DK * o
            h_sb = []
            for f in range(FK):
                psA = abpsum.tile([P, tcz], dt.float32, tag="psA")
                for k in range(DK):
                    nc.tensor.matmul(
                        psA[:], lhsT=w1_sb[:, f * BLK + k * P:f * BLK + (k + 1) * P],
                        rhs=x_sb[:, xo + k * tcz:xo + (k + 1) * tcz],
                        start=(k == 0), stop=(k == DK - 1))
                psB = abpsum.tile([P, tcz], dt.float32, tag="psB")
                for k in range(DK):
                    nc.tensor.matmul(
                        psB[:], lhsT=w3_sb[:, f * BLK + k * P:f * BLK + (k + 1) * P],
                        rhs=x_sb[:, xo + k * tcz:xo + (k + 1) * tcz],
                        start=(k == 0), stop=(k == DK - 1))
                ssb = spool.tile([P, tcz], dt.bfloat16, tag="ssb")
                nc.scalar.activation(ssb[:], psA[:], AF.Silu)
                hsb = hpool.tile([P, tcz], dt.bfloat16, tag=f"h{f}")
                nc.vector.tensor_tensor(hsb[:], ssb[:], psB[:], op=ALU.mult)
                h_sb.append(hsb)
            # Y-stage, w2-stationary: psY[dt] = sum_fk w2T[fk, dtile] @ h[fk]
            # fkh-outer so the first half only needs w2 cols 0..4D
            for dhalf in range(2):
                psY = [ypsum.tile([P, tcz], dt.float32, tag="psY",
                                  name=f"psY{dhalf}_{i}") for i in range(4)]
                for fkh in range(2):
                    for dt_ in range(4):
                        dglob = dhalf * 4 + dt_
                        for fk in range(fkh * 4, fkh * 4 + 4):
                            nc.tensor.matmul(
                                psY[dt_][:],
                                lhsT=w2_sb[:, fk * D + dglob * P:fk * D + dglob * P + P],
                                rhs=h_sb[fk][:],
                                start=(fk == 0), stop=(fk == FK - 1))
                tail = (o + tcz == cap) and dhalf == 1
                for dt_ in range(4):
                    dglob = dhalf * 4 + dt_
                    if tail:
                        # final chain: halve each copy across both engines so
                        # the post-last-matmul latency is a half-width copy
                        hw_ = tcz // 2
                        ysb = ypool.tile([P, tcz], dt.bfloat16, tag="ysb")
                        nc.scalar.activation(ysb[:, 0:hw_], psY[dt_][:, 0:hw_],
                                             AF.Copy)
                        nc.vector.tensor_scalar_mul(ysb[:, hw_:tcz],
                                                    psY[dt_][:, hw_:tcz], 1.0)
                    elif dt_ % 2 == 0:
                        # alternate copy engines: two parallel copy streams
                        ysb = ypool.tile([P, tcz], dt.bfloat16, tag="ysb")
                        nc.scalar.activation(ysb[:], psY[dt_][:], AF.Copy)
                    else:
                        ysb = ypool.tile([P, tcz], dt.bfloat16, tag="ysbv")
                        nc.vector.tensor_scalar_mul(ysb[:], psY[dt_][:], 1.0)
                    nc.sync.dma_start(out[dglob * P:(dglob + 1) * P, o:o + tcz],
                                      ysb[:])

    nc.compile()
    return nc


def _route(xf, gate_w):
    """Host gate: returns per-expert (token indices, renormalized weights)."""
    logits = xf.astype(np.float64) @ gate_w.astype(np.float64).T   # [T, E]
    order = np.argsort(-logits, axis=1, kind="stable")
    i1 = order[:, 0]
    i2 = order[:, 1]
    ar = np.arange(T)
    l1 = logits[ar, i1]
    l2 = logits[ar, i2]
    g1 = 1.0 / (1.0 + np.exp(l2 - l1))
    g2 = 1.0 - g1
    idx_e, scl_e = [], []
    for e in range(E):
        m1 = i1 == e
        m2 = i2 == e
        ids = np.concatenate([np.nonzero(m1)[0], np.nonzero(m2)[0]])
        sc = np.concatenate([g1[m1], g2[m2]])
        idx_e.append(ids)
        scl_e.append(sc.astype(np.float32))
    return idx_e, scl_e


def prepare(x, gate_w, w1, w3, w2):
    """Host routing + sharding: returns (nc, in_maps, (idx_e, scl_e))."""
    import ml_dtypes

    xf = np.ascontiguousarray(x.reshape(T, D).astype(np.float32))
    xTb = np.ascontiguousarray(xf.T).astype(ml_dtypes.bfloat16)   # [D, T]

    idx_e, scl_e = _route(xf, gate_w)
    maxcnt = max(len(i) for i in idx_e)
    cap = ((maxcnt + 3) // 4) * 4     # 4-token grain keeps DMA rows 8B-aligned
    chunks = _chunks(cap)

    if cap not in _cache:
        _cache[cap] = _build_nc(cap)
    nc = _cache[cap]

    in_maps = []
    for c in range(NCORES):
        ids = idx_e[c]
        cnt = len(ids)
        xg = np.zeros((D, cap), dtype=ml_dtypes.bfloat16)
        xg[:, :cnt] = xTb[:, ids]
        # chunk-major, then k-major partition blocks: chunk rows contiguous
        xh = np.concatenate([
            xg[:, o:o + tcz].reshape(DK, P, tcz).transpose(1, 0, 2)
            .reshape(P, DK * tcz) for (o, tcz) in chunks], axis=1)

        w1T = np.ascontiguousarray(w1[c].T).astype(ml_dtypes.bfloat16)  # [D,F]
        w3T = np.ascontiguousarray(w3[c].T).astype(ml_dtypes.bfloat16)
        w2T = np.ascontiguousarray(w2[c].T).astype(ml_dtypes.bfloat16)  # [F,D]

        def fmaj(wT):
            # [D, F] -> [128, f-major [f][k][128]] per-f-tile blocks
            return np.concatenate([
                wT[:, f * P:(f + 1) * P].reshape(DK, P, P).transpose(1, 0, 2)
                .reshape(P, BLK) for f in range(FK)], axis=1)

        in_maps.append({
            "xh": np.ascontiguousarray(xh),
            "w1h": fmaj(w1T),
            "w3h": fmaj(w3T),
            "w2h": np.ascontiguousarray(
                w2T.reshape(FK, P, D).transpose(1, 0, 2).reshape(P, FK * D)),
        })
    return nc, in_maps, (idx_e, scl_e)


def _combine(res, meta):
    idx_e, scl_e = meta
    outf = np.zeros((T, D), dtype=np.float32)
    for c in range(NCORES):
        cnt = len(idx_e[c])
        y = res.results[c]["out"][:, :cnt].astype(np.float32).T   # [cnt, D]
        outf[idx_e[c]] += scl_e[c][:, None] * y
    return outf.reshape(B, S, D)


def kernel(x, gate_w, w1, w3, w2):
    from concourse.bass_utils import run_bass_kernel_spmd

    nc, in_maps, meta = prepare(x, gate_w, w1, w3, w2)
    res = run_bass_kernel_spmd(nc, in_maps, list(range(NCORES)))
    return _combine(res, meta)


# revision 14
# speedup vs baseline: 1.0105x; 1.0105x over previous
"""Grouped MoE (top-2 of 8 experts, SwiGLU) on 8 Trainium2 NeuronCores.

Expert-parallel with host routing (gate on host, exact). Core c owns
expert c; tokens are gathered per expert into a fixed-capacity [D, cap]
buffer. On device each core runs the three SwiGLU GEMMs in bf16 over its
~T*K/E tokens and writes an UNSCALED output in transposed [D, cap]
layout; the host applies the per-token gate weight and scatter-adds the
two expert contributions. No collectives.

v2 layout/schedule changes vs the first working kernel:
 - All DRAM inputs are packed partition-major so every DMA moves 2-16 KB
   contiguous rows (128 descriptors/transfer, full HBM rate). w1/w3 are
   packed in per-f-tile blocks so the first A-stage matmul only needs
   x-chunk0 + one 256 KB block instead of ~2 MB.
 - Y-stage is w2-stationary (output [D, tokens]): no partial m-tiles,
   arbitrary chunk sizes, and the gate scale moves to the host combine.
 - First/last chunks are small to shrink the DMA lead-in and the
   copy+store tail after the final matmul.
"""

import sys
import numpy as np

for _p in ("/opt/trn_rl_repo",):
    if _p not in sys.path:
        sys.path.insert(0, _p)

B, S, D, F, E, K = 2, 2048, 1024, 1024, 8, 2
T = B * S            # 4096 tokens
NCORES = 8
P = 128
DK = D // P          # 8 contraction chunks over D
FK = F // P          # 8 F tiles
BLK = DK * P         # w1/w3 f-block stride (k-major within a block)
NWARM = 12           # PE warm-up matmuls while the first DMAs land

_cache = {}


def _chunks(cap):
    """Token chunks <= 512 (PSUM bank limit), first/last kept small-ish.

    A/B and Y matmul cost is proportional to total tokens for any chunk
    >= ~192 (LDWEIGHTS hides under the column stream), so only the first
    chunk (gates the DMA lead-in) and last chunk (gates the tail) matter.
    """
    if cap <= 512:
        sizes = [cap]
    elif cap <= 832:
        sizes = [(cap + 1) // 2, cap // 2]
    else:
        sizes = [320]
        rem = cap - 320
        while rem > 704:
            sizes.append(512)
            rem -= 512
        if rem > 512:
            sizes += [(rem + 1) // 2, rem // 2]
        else:
            sizes.append(rem)
    out = []
    o = 0
    for s in sizes:
        out.append((o, s))
        o += s
    assert o == cap and all(0 < s <= 512 for _, s in out)
    return out


def _build_nc(cap):
    from contextlib import ExitStack

    import concourse.mybir as mybir
    import concourse.tile as tile
    from concourse import bacc

    dt = mybir.dt
    AF = mybir.ActivationFunctionType
    ALU = mybir.AluOpType

    chunks = _chunks(cap)

    nc = bacc.Bacc("TRN2", target_bir_lowering=False, debug=False,
                   num_devices=NCORES)

    # all partition-major: row p holds that partition's full data span
    xh = nc.dram_tensor("xh", [P, DK * cap], dt.bfloat16,
                        kind="ExternalInput").ap()
    w1h = nc.dram_tensor("w1h", [P, FK * BLK], dt.bfloat16,
                         kind="ExternalInput").ap()
    w3h = nc.dram_tensor("w3h", [P, FK * BLK], dt.bfloat16,
                         kind="ExternalInput").ap()
    w2h = nc.dram_tensor("w2h", [P, FK * D], dt.bfloat16,
                         kind="ExternalInput").ap()
    out = nc.dram_tensor("out", [D, cap], dt.bfloat16,
                         kind="ExternalOutput").ap()

    with tile.TileContext(nc) as tc, ExitStack() as ctx:
        const = ctx.enter_context(tc.tile_pool(name="const", bufs=1))
        xpool = ctx.enter_context(tc.tile_pool(name="xpool", bufs=1))
        spool = ctx.enter_context(tc.tile_pool(name="spool", bufs=2))
        hpool = ctx.enter_context(tc.tile_pool(name="hpool", bufs=2))
        ypool = ctx.enter_context(tc.tile_pool(name="ypool", bufs=3))

        abpsum = ctx.enter_context(tc.tile_pool(name="abpsum", bufs=2,
                                                space="PSUM"))
        ypsum = ctx.enter_context(tc.tile_pool(name="ypsum", bufs=4,
                                               space="PSUM"))

        x_sb = xpool.tile([P, DK * cap], dt.bfloat16, tag="xall")
        w1_sb = const.tile([P, FK * BLK], dt.bfloat16, tag="w1")
        w3_sb = const.tile([P, FK * BLK], dt.bfloat16, tag="w3")
        w2_sb = const.tile([P, FK * D], dt.bfloat16, tag="w2")

        # ---- DMA issue order = criticality order. Every transfer below is
        # 128 descriptors of >= 2KB contiguous rows (full HBM rate).
        # First A-group (f=0) is gated only on x-chunk0 + w1/w3 f0 blocks
        # (~1 MB); remaining f-blocks stream in ahead of the compute. ----
        # All DMA on the sync queue: its sequencer starts issuing at ~0.1us
        # while the other engines only come online at 5-8us, so anything
        # issued elsewhere would land BEHIND these transfers in the rings.
        o0, tc0 = chunks[0]
        nc.sync.dma_start(x_sb[:, 0:DK * tc0], xh[:, 0:DK * tc0])
        nc.sync.dma_start(w1_sb[:, 0:BLK], w1h[:, 0:BLK])
        nc.sync.dma_start(w3_sb[:, 0:BLK], w3h[:, 0:BLK])
        for f in range(1, FK):
            nc.sync.dma_start(w1_sb[:, f * BLK:(f + 1) * BLK],
                              w1h[:, f * BLK:(f + 1) * BLK])
            nc.sync.dma_start(w3_sb[:, f * BLK:(f + 1) * BLK],
                              w3h[:, f * BLK:(f + 1) * BLK])
        # w2 in fk-halves; the Y loop consumes fk 0..3 before 4..7
        nc.sync.dma_start(w2_sb[:, 0:4 * D], w2h[:, 0:4 * D])
        nc.sync.dma_start(w2_sb[:, 4 * D:8 * D], w2h[:, 4 * D:8 * D])
        for (o, tcz) in chunks[1:]:
            nc.sync.dma_start(x_sb[:, DK * o:DK * (o + tcz)],
                              xh[:, DK * o:DK * (o + tcz)])

        # ---- PE warm-up: dummy matmuls while the first DMAs land keep the
        # HAM activity window full so the PE reaches max p-state ----
        wrm = spool.tile([P, 512], dt.bfloat16, tag="wrm")
        nc.vector.memset(wrm[:], 0.5)
        for _ in range(NWARM):
            psW = abpsum.tile([P, 512], dt.float32, tag="psA", name="psW")
            nc.tensor.matmul(psW[:], lhsT=wrm[:, 0:P], rhs=wrm[:],
                             start=True, stop=True)

        # ---- per-chunk SwiGLU FFN ----
        for (o, tcz) in chunks:
            xo = DK * o
            h_sb = []
            for f in range(FK):
                psA = abpsum.tile([P, tcz], dt.float32, tag="psA")
                for k in range(DK):
                    nc.tensor.matmul(
                        psA[:], lhsT=w1_sb[:, f * BLK + k * P:f * BLK + (k + 1) * P],
                        rhs=x_sb[:, xo + k * tcz:xo + (k + 1) * tcz],
                        start=(k == 0), stop=(k == DK - 1))
                psB = abpsum.tile([P, tcz], dt.float32, tag="psB")
                for k in range(DK):
                    nc.tensor.matmul(
                        psB[:], lhsT=w3_sb[:, f * BLK + k * P:f * BLK + (k + 1) * P],
                        rhs=x_sb[:, xo + k * tcz:xo + (k + 1) * tcz],
                        start=(k == 0), stop=(k == DK - 1))
                ssb = spool.tile([P, tcz], dt.bfloat16, tag="ssb")
                nc.scalar.activation(ssb[:], psA[:], AF.Silu)
                hsb = hpool.tile([P, tcz], dt.bfloat16, tag=f"h{f}")
                nc.vector.tensor_tensor(hsb[:], ssb[:], psB[:], op=ALU.mult)
                h_sb.append(hsb)
            # Y-stage, w2-stationary: psY[dt] = sum_fk w2T[fk, dtile] @ h[fk]
            # fkh-outer so the first half only needs w2 cols 0..4D
            for dhalf in range(2):
                psY = [ypsum.tile([P, tcz], dt.float32, tag="psY",
                                  name=f"psY{dhalf}_{i}") for i in range(4)]
                for fkh in range(2):
                    for dt_ in range(4):
                        dglob = dhalf * 4 + dt_
                        for fk in range(fkh * 4, fkh * 4 + 4):
                            nc.tensor.matmul(
                                psY[dt_][:],
                                lhsT=w2_sb[:, fk * D + dglob * P:fk * D + dglob * P + P],
                                rhs=h_sb[fk][:],
                                start=(fk == 0), stop=(fk == FK - 1))
                for dt_ in range(4):
                    dglob = dhalf * 4 + dt_
                    # alternate copy engines: two parallel copy streams
                    if dt_ % 2 == 0:
                        ysb = ypool.tile([P, tcz], dt.bfloat16, tag="ysb")
                        nc.scalar.activation(ysb[:], psY[dt_][:], AF.Copy)
                    else:
                        ysb = ypool.tile([P, tcz], dt.bfloat16, tag="ysbv")
                        nc.vector.tensor_scalar_mul(ysb[:], psY[dt_][:], 1.0)
                    nc.sync.dma_start(out[dglob * P:(dglob + 1) * P, o:o + tcz],
                                      ysb[:])

    nc.compile()
    return nc


def _route(xf, gate_w):
    """Host gate: returns per-expert (token indices, renormalized weights)."""
    logits = xf.astype(np.float64) @ gate_w.astype(np.float64).T   # [T, E]
    order = np.argsort(-logits, axis=1, kind="stable")
    i1 = order[:, 0]
    i2 = order[:, 1]
    ar = np.arange(T)
    l1 = logits[ar, i1]
    l2 = logits[ar, i2]
    g1 = 1.0 / (1.0 + np.exp(l2 - l1))
    g2 = 1.0 - g1
    idx_e, scl_e = [], []
    for e in range(E):
        m1 = i1 == e
        m2 = i2 == e
        ids = np.concatenate([np.nonzero(m1)[0], np.nonzero(m2)[0]])
        sc = np.concatenate([g1[m1], g2[m2]])
        idx_e.append(ids)
        scl_e.append(sc.astype(np.float32))
    return idx_e, scl_e


def prepare(x, gate_w, w1, w3, w2):
    """Host routing + sharding: returns (nc, in_maps, (idx_e, scl_e))."""
    import ml_dtypes

    xf = np.ascontiguousarray(x.reshape(T, D).astype(np.float32))
    xTb = np.ascontiguousarray(xf.T).astype(ml_dtypes.bfloat16)   # [D, T]

    idx_e, scl_e = _route(xf, gate_w)
    maxcnt = max(len(i) for i in idx_e)
    cap = ((maxcnt + 3) // 4) * 4     # 4-token grain keeps DMA rows 8B-aligned
    chunks = _chunks(cap)

    if cap not in _cache:
        _cache[cap] = _build_nc(cap)
    nc = _cache[cap]

    in_maps = []
    for c in range(NCORES):
        ids = idx_e[c]
        cnt = len(ids)
        xg = np.zeros((D, cap), dtype=ml_dtypes.bfloat16)
        xg[:, :cnt] = xTb[:, ids]
        # chunk-major, then k-major partition blocks: chunk rows contiguous
        xh = np.concatenate([
            xg[:, o:o + tcz].reshape(DK, P, tcz).transpose(1, 0, 2)
            .reshape(P, DK * tcz) for (o, tcz) in chunks], axis=1)

        w1T = np.ascontiguousarray(w1[c].T).astype(ml_dtypes.bfloat16)  # [D,F]
        w3T = np.ascontiguousarray(w3[c].T).astype(ml_dtypes.bfloat16)
        w2T = np.ascontiguousarray(w2[c].T).astype(ml_dtypes.bfloat16)  # [F,D]

        def fmaj(wT):
            # [D, F] -> [128, f-major [f][k][128]] per-f-tile blocks
            return np.concatenate([
                wT[:, f * P:(f + 1) * P].reshape(DK, P, P).transpose(1, 0, 2)
                .reshape(P, BLK) for f in range(FK)], axis=1)

        in_maps.append({
            "xh": np.ascontiguousarray(xh),
            "w1h": fmaj(w1T),
            "w3h": fmaj(w3T),
            "w2h": np.ascontiguousarray(
                w2T.reshape(FK, P, D).transpose(1, 0, 2).reshape(P, FK * D)),
        })
    return nc, in_maps, (idx_e, scl_e)


def _combine(res, meta):
    idx_e, scl_e = meta
    outf = np.zeros((T, D), dtype=np.float32)
    for c in range(NCORES):
        cnt = len(idx_e[c])
        y = res.results[c]["out"][:, :cnt].astype(np.float32).T   # [cnt, D]
        outf[idx_e[c]] += scl_e[c][:, None] * y
    return outf.reshape(B, S, D)


def kernel(x, gate_w, w1, w3, w2):
    from concourse.bass_utils import run_bass_kernel_spmd

    nc, in_maps, meta = prepare(x, gate_w, w1, w3, w2)
    res = run_bass_kernel_spmd(nc, in_maps, list(range(NCORES)))
    return _combine(res, meta)
